# revision 19
# baseline (speedup 1.0000x reference)
"""Trainium2 Bass kernel for AbLang2-style MHA (B=2, N=2048, H=16, dh=64).

Sharding: 8 cores = 2 batches x 4 head-groups (4 heads/core).
Per core, everything is in "transposed" layout where convenient:
  - qT/kT computed as [qcol, n] (qcol on partitions) so the scores matmul
    needs no transposes;  v computed in natural [m, vcol] layout.
  - scoresT is [m(keys), n(queries)]: softmax denominator comes for free as
    an extra ones-column appended to V (one more output row of the attn@V
    matmul), and the padding mask is a free per-partition bias on the Exp.
  - normalization (1/rowsum) applied AFTER attn@V on the tiny [65,2048]
    per-head output via a rank-4 broadcast matmul + one tensor_mul.
  - out-projection produces a per-core partial [2048,1024]; host sums the
    4 partials per batch and adds bo.
Bias folding: q scaling (0.125 * 1/sqrt(64) = 1/64) folded into Wq/bq on
host; biases folded into the matmuls via a ones-row appended to xT.
"""

import os
import sys

import numpy as np

sys.path.insert(0, "/opt/trn_rl_repo")

N = 2048
D = 1024
HC = 4          # heads per core
DH = 64
QC = HC * DH    # 256 qcols per core
NT = N // 128   # 16 tiles of 128
KC = 9          # 8 dmodel chunks of 128 + 1 ones-row chunk
VW = HC * (DH + 1)  # 260: v with a ones col per head

USE_F32R = os.environ.get("KERNEL_F32", "") != "1"
PHASES = os.environ.get("KERNEL_PHASES", "123")

_CACHE = {}


def _build_nc():
    import concourse.bass as bass
    import concourse.bacc as bacc
    import concourse.mybir as mybir
    from concourse import tile
    from contextlib import ExitStack

    f32 = mybir.dt.float32
    AF = mybir.ActivationFunctionType

    def mm(ap):
        return ap.bitcast(mybir.dt.float32r) if USE_F32R else ap

    mo = mm  # producers feeding f32r matmuls must emit f32r-typed output

    nc = bacc.Bacc("TRN2", target_bir_lowering=False, debug=False)
    xT = nc.declare_dram_parameter("xT", [1025, N], f32, isOutput=False)
    wqT = nc.declare_dram_parameter("wqT", [1025, QC], f32, isOutput=False)
    wkT = nc.declare_dram_parameter("wkT", [1025, QC], f32, isOutput=False)
    wvT = nc.declare_dram_parameter("wvT", [1025, VW], f32, isOutput=False)
    woT = nc.declare_dram_parameter("woT", [QC, D], f32, isOutput=False)
    cosT = nc.declare_dram_parameter("cosT", [128, N], f32, isOutput=False)
    sinT = nc.declare_dram_parameter("sinT", [128, N], f32, isOutput=False)
    rmat = nc.declare_dram_parameter("rmat", [128, 128], f32, isOutput=False)
    maskT = nc.declare_dram_parameter("maskT", [128, NT], f32, isOutput=False)
    ind0 = nc.declare_dram_parameter("ind0", [128, 128], f32, isOutput=False)
    ind1 = nc.declare_dram_parameter("ind1", [128, 128], f32, isOutput=False)
    out_part = nc.declare_dram_parameter("out_part", [N, D], f32, isOutput=True)

    with ExitStack() as ctx:
        tc = ctx.enter_context(tile.TileContext(nc))
        const = ctx.enter_context(tc.tile_pool(name="const", bufs=1))

        # ---- constants / weights into SBUF ----
        wq_sb = const.tile([128, KC * QC], f32, tag="wq")
        wk_sb = const.tile([128, KC * QC], f32, tag="wk")
        wv_sb = const.tile([128, KC * VW], f32, tag="wv")
        for (sb, dr, w) in ((wv_sb, wvT, VW), (wk_sb, wkT, QC), (wq_sb, wqT, QC)):
            nc.sync.dma_start(
                mo(sb[:, 0:8 * w].rearrange("p (c q) -> p c q", q=w)),
                mo(dr[0:1024, :].rearrange("(c p) q -> p c q", p=128)))
            nc.sync.dma_start(mo(sb[0:1, 8 * w:9 * w]), mo(dr[1024:1025, :]))
        wo_sb = const.tile([128, 2 * D], f32, tag="wo")
        nc.sync.dma_start(
            mo(wo_sb[:].rearrange("p (c d) -> p c d", d=D)),
            mo(woT[:].rearrange("(c p) d -> p c d", p=128)))
        cos_sb = const.tile([128, N], f32, tag="cos")
        sin_sb = const.tile([128, N], f32, tag="sin")
        r_sb = const.tile([128, 128], f32, tag="rm")
        mask_sb = const.tile([128, NT], f32, tag="mask")
        i0_sb = const.tile([128, 128], f32, tag="i0")
        i1_sb = const.tile([128, 128], f32, tag="i1")
        nc.sync.dma_start(cos_sb[:], cosT[:])
        nc.sync.dma_start(sin_sb[:], sinT[:])
        nc.sync.dma_start(mo(r_sb[:]), mo(rmat[:]))
        nc.sync.dma_start(mask_sb[:], maskT[:])
        nc.sync.dma_start(mo(i0_sb[:]), mo(ind0[:]))
        nc.sync.dma_start(mo(i1_sb[:]), mo(ind1[:]))

        # persistent activations
        act = ctx.enter_context(tc.tile_pool(name="act", bufs=1))
        qrope = [act.tile([128, N], f32, tag=f"qr{t}", name=f"qr{t}") for t in range(2)]
        krope = [act.tile([128, N], f32, tag=f"kr{t}", name=f"kr{t}") for t in range(2)]
        v_sb = [act.tile([128, VW], f32, tag=f"v{j}", name=f"v{j}") for j in range(NT)]

        # ================= Phase 1: QKV projections + RoPE =================
        if "1" in PHASES:
         with ExitStack() as p1:
            xpool = p1.enter_context(tc.tile_pool(name="xt", bufs=2))
            qbp = p1.enter_context(tc.tile_pool(name="qb", bufs=4))
            tmpp = p1.enter_context(tc.tile_pool(name="tmp", bufs=4))
            psq = p1.enter_context(tc.tile_pool(name="psq", bufs=3, space="PSUM"))
            psr = p1.enter_context(tc.tile_pool(name="psr", bufs=2, space="PSUM"))
            psv = p1.enter_context(tc.tile_pool(name="psv", bufs=2, space="PSUM"))
            for ci in range(4):  # n-chunks of 512
                ns = slice(ci * 512, (ci + 1) * 512)
                xt = xpool.tile([128, KC * 512], f32, tag="xt")
                nc.sync.dma_start(
                    mo(xt[:, 0:8 * 512].rearrange("p (c n) -> p c n", n=512)),
                    mo(xT[0:1024, ns].rearrange("(c p) n -> p c n", p=128)))
                nc.sync.dma_start(mo(xt[0:1, 8 * 512:9 * 512]), mo(xT[1024:1025, ns]))

                for (w, rope) in ((wq_sb, qrope), (wk_sb, krope)):
                    for t in range(2):
                        ps = psq.tile([128, 512], f32, tag="ps")
                        for c in range(KC):
                            kp = 128 if c < 8 else 1
                            nc.tensor.matmul(
                                ps[:],
                                mm(w[0:kp, c * QC + t * 128:c * QC + (t + 1) * 128]),
                                mm(xt[0:kp, c * 512:c * 512 + 512]),
                                start=(c == 0), stop=(c == KC - 1))
                        qb = qbp.tile([128, 512], f32, tag="qb")
                        nc.scalar.copy(mo(qb[:]), ps[:])
                        rp = psr.tile([128, 512], f32, tag="rp")
                        nc.tensor.matmul(rp[:], mm(r_sb[:]), mm(qb[:]), start=True, stop=True)
                        t1 = tmpp.tile([128, 512], f32, tag="t1")
                        t2 = tmpp.tile([128, 512], f32, tag="t2")
                        nc.vector.tensor_mul(t1[:], qb[:], cos_sb[:, ns])
                        nc.vector.tensor_mul(t2[:], rp[:], sin_sb[:, ns])
                        nc.vector.tensor_add(mo(rope[t][:, ns]), t1[:], t2[:])

                for j in range(4):  # m-tiles within this n-chunk
                    pv = psv.tile([128, VW], f32, tag="pv")
                    for c in range(KC):
                        kp = 128 if c < 8 else 1
                        nc.tensor.matmul(
                            pv[:],
                            mm(xt[0:kp, c * 512 + j * 128:c * 512 + (j + 1) * 128]),
                            mm(wv_sb[0:kp, c * VW:(c + 1) * VW]),
                            start=(c == 0), stop=(c == KC - 1))
                    nc.vector.tensor_copy(mo(v_sb[ci * 4 + j][:]), pv[:])

        # ================= Phase 2: attention per head =================
        outp = ctx.enter_context(tc.tile_pool(name="outp", bufs=1))
        out_sb = [outp.tile([65, N], f32, tag=f"o{h}", name=f"o{h}") for h in range(HC)]
        if "2" in PHASES:
         with ExitStack() as p2:
            spool = p2.enter_context(tc.tile_pool(name="sc", bufs=2, space="PSUM"))
            opool = p2.enter_context(tc.tile_pool(name="ov", bufs=1, space="PSUM"))
            apool = p2.enter_context(tc.tile_pool(name="at", bufs=4))
            for h in range(HC):
                t, half = h // 2, (h % 2) * 64
                ops = opool.tile([65, N], f32, tag="ops")
                for cn in range(2):  # n-chunks of 1024
                    for m in range(NT):
                        sps = spool.tile([128, 1024], f32, tag="sps")
                        for s in range(2):
                            nc.tensor.matmul(
                                sps[:, s * 512:(s + 1) * 512],
                                mm(krope[t][half:half + 64, m * 128:(m + 1) * 128]),
                                mm(qrope[t][half:half + 64, cn * 1024 + s * 512:cn * 1024 + (s + 1) * 512]),
                                start=True, stop=True)
                        at = apool.tile([128, 1024], f32, tag="at")
                        nc.scalar.activation(mo(at[:]), sps[:], AF.Exp, bias=mask_sb[:, m:m + 1])
                        for s in range(2):
                            nc.tensor.matmul(
                                ops[0:65, cn * 1024 + s * 512:cn * 1024 + (s + 1) * 512],
                                mm(v_sb[m][:, h * 65:(h + 1) * 65]),
                                mm(at[:, s * 512:(s + 1) * 512]),
                                start=(m == 0), stop=(m == NT - 1))
                nc.vector.tensor_copy(out_sb[h][:], ops[:])

        # ================= Phase 3: normalize + out-projection =================
        if "3" in PHASES:
         with ExitStack() as p3:
            rpool = p3.enter_context(tc.tile_pool(name="rc", bufs=1))
            onp = p3.enter_context(tc.tile_pool(name="on", bufs=1))
            bps = opool
            fps = spool
            den4 = rpool.tile([128, N], f32, tag="den")
            recip4 = rpool.tile([128, N], f32, tag="recip")
            nc.gpsimd.memset(den4[:], 1.0)
            for h in range(HC):
                nc.scalar.copy(den4[32 * h:32 * h + 1, :], out_sb[h][64:65, :])
            with nc.allow_low_precision(reason="f32r rounding for matmul feed"):
                for rc in range(4):
                    rs = slice(rc * 512, (rc + 1) * 512)
                    nc.vector.reciprocal(mo(recip4[:, rs]), den4[:, rs])
            onorm = [[onp.tile([128, 512], f32, tag=f"on{p}_{c4}", name=f"on{p}_{c4}")
                      for c4 in range(4)] for p in range(2)]
            for c4 in range(4):
                cs = slice(c4 * 512, (c4 + 1) * 512)
                for p, isb in ((0, i0_sb), (1, i1_sb)):
                    bp = bps.tile([128, 512], f32, tag="ops", name="bp")
                    nc.tensor.matmul(bp[:], mm(isb[:]), mm(recip4[:, cs]), start=True, stop=True)
                    for hh in range(2):
                        h = 2 * p + hh
                        nc.vector.tensor_mul(
                            mo(onorm[p][c4][hh * 64:(hh + 1) * 64, :]),
                            out_sb[h][0:64, cs], bp[hh * 64:(hh + 1) * 64, :])
            fsb = p3.enter_context(tc.tile_pool(name="fsb", bufs=4))
            for j in range(NT):
                fp = fps.tile([128, D], f32, tag="sps", name="fp")
                for dc in range(2):
                    for p in range(2):
                        nc.tensor.matmul(
                            fp[:, dc * 512:(dc + 1) * 512],
                            mm(onorm[p][j // 4][:, (j % 4) * 128:(j % 4 + 1) * 128]),
                            mm(wo_sb[:, p * D + dc * 512:p * D + (dc + 1) * 512]),
                            start=(p == 0), stop=(p == 1))
                fs = fsb.tile([128, D], f32, tag="fs")
                nc.vector.tensor_copy(fs[:], fp[:])
                nc.sync.dma_start(out_part[j * 128:(j + 1) * 128, :], fs[:])
    nc.compile()
    return nc


def _prep_inputs(x, padding_mask, Wq, bq, Wk, bk, Wv, bv, Wo, bo, freqs):
    f = np.float32
    x = np.asarray(x, f)
    pos = np.arange(N, dtype=f)
    ang = np.repeat(np.outer(pos, np.asarray(freqs, f)), 2, axis=1)  # [N, 64]
    cosT = np.ascontiguousarray(np.concatenate([np.cos(ang).T] * 2, axis=0), dtype=f)
    sinT = np.ascontiguousarray(np.concatenate([np.sin(ang).T] * 2, axis=0), dtype=f)
    rmat = np.zeros((128, 128), f)
    for i in range(64):
        rmat[2 * i + 1, 2 * i] = -1.0
        rmat[2 * i, 2 * i + 1] = 1.0
    ind = np.zeros((2, 128, 128), f)
    ind[0, 0, 0:64] = 1.0
    ind[0, 32, 64:128] = 1.0
    ind[1, 64, 0:64] = 1.0
    ind[1, 96, 64:128] = 1.0
    xTs, masks = [], []
    for b in range(2):
        xt = np.empty((1025, N), f)
        xt[:1024] = x[b].T
        xt[1024] = 1.0
        xTs.append(xt)
        mb = np.where(np.asarray(padding_mask[b], bool), f(-10000.0), f(0.0)).astype(f)
        masks.append(np.ascontiguousarray(mb.reshape(NT, 128).T))
    Wq = np.asarray(Wq, f) / 64.0
    bqs = np.asarray(bq, f) / 64.0
    Wk, bk = np.asarray(Wk, f), np.asarray(bk, f)
    Wv, bv = np.asarray(Wv, f), np.asarray(bv, f)
    Wo = np.asarray(Wo, f)
    in_maps = []
    for core in range(8):
        b, hs = core // 4, core % 4
        sl = slice(hs * QC, (hs + 1) * QC)
        wqt = np.empty((1025, QC), f)
        wqt[:1024] = Wq[sl, :].T
        wqt[1024] = bqs[sl]
        wkt = np.empty((1025, QC), f)
        wkt[:1024] = Wk[sl, :].T
        wkt[1024] = bk[sl]
        wvt = np.zeros((1025, VW), f)
        for h in range(HC):
            wvt[:1024, h * 65:h * 65 + 64] = Wv[hs * QC + h * 64:hs * QC + (h + 1) * 64, :].T
            wvt[1024, h * 65:h * 65 + 64] = bv[hs * QC + h * 64:hs * QC + (h + 1) * 64]
            wvt[1024, h * 65 + 64] = 1.0
        in_maps.append({
            "xT": xTs[b], "wqT": wqt, "wkT": wkt, "wvT": wvt,
            "woT": np.ascontiguousarray(Wo[:, sl].T),
            "cosT": cosT, "sinT": sinT, "rmat": rmat, "maskT": masks[b],
            "ind0": ind[0], "ind1": ind[1],
        })
    return in_maps


def kernel(x, padding_mask, Wq, bq, Wk, bk, Wv, bv, Wo, bo, freqs):
    in_maps = _prep_inputs(x, padding_mask, Wq, bq, Wk, bk, Wv, bv, Wo, bo, freqs)
    if "nc" not in _CACHE:
        _CACHE["nc"] = _build_nc()
    nc = _CACHE["nc"]
    from concourse.bass_utils import run_bass_kernel_spmd
    trace = os.environ.get("KERNEL_TRACE", "") == "1"
    res = run_bass_kernel_spmd(nc, in_maps, list(range(8)), trace=trace)
    if os.environ.get("KERNEL_TIME", "") == "1":
        import time as _time
        best = None
        for _ in range(4):
            t0 = _time.perf_counter()
            res = run_bass_kernel_spmd(nc, in_maps, list(range(8)), trace=False)
            dt = _time.perf_counter() - t0
            best = dt if best is None else min(best, dt)
            print(f"  spmd call wall: {dt*1e6:.0f} us", file=sys.stderr)
        print(f"KERNEL best wall: {best*1e6:.0f} us", file=sys.stderr)
    if trace:
        print(f"KERNEL exec_time_ns: {res.exec_time_ns}", file=sys.stderr)
        _CACHE["last_result"] = res
    parts = [r["out_part"] for r in res.results]
    bo = np.asarray(bo, np.float32)
    out = np.stack([parts[0] + parts[1] + parts[2] + parts[3] + bo,
                    parts[4] + parts[5] + parts[6] + parts[7] + bo])
    return out.astype(np.float32)
